# revision 29
# baseline (speedup 1.0000x reference)
"""Bahdanau additive-attention kernel for Trainium2, SPMD across 8 NeuronCores.

Reference computation (all fp32):
    q_proj  = query @ W1_w.T + W1_b            # [D]
    v_proj  = values @ W2_w.T + W2_b           # [T, D]
    weights = softmax(tanh(q_proj + v_proj) * v, axis=0)   # over T
    out     = weights * values                 # [T, D]

Sharding: values is split along T across 8 cores (2048 rows each); W2 is
replicated (shipped pre-transposed + pre-blocked in bf16); the q-projection
matvec is sharded over the contraction dim (each core handles 256 columns of
W1) and finished with an AllReduce; the softmax denominator (per-column sum
of exps) is AllReduced.  Logits are bounded in [-0.1, 0.1] (tanh * v with
|v| <= 0.1) so the softmax needs no max-subtraction pass.

Per-core device program:
  - VT (values shard transposed, bf16) resident in SBUF as the moving matmul
    operand; psum tiles are [d=128 part, t=512 free]; the k loop is OUTER so
    the first tiles stream at DMA pace and the stationary operand is reused
    across 4 consecutive matmuls.
  - ScalarE: tanh(psum + qb[d]) then exp(v[d] * x) with accum_out giving the
    per-partition running sum of exps (softmax denominator) for free.
  - e stored fp16 in SBUF.  Pass 2: e *= 1/S[d] (per-partition tensor_scalar,
    in place), outT = e * valuesT(fp32) on DVE, TensorE transposes outT back
    to [t, d], ScalarE evacuates PSUM to SBUF, DMA out.
"""

import numpy as np

import concourse.bacc as bacc
import concourse.bass as bass
import concourse.tile as tile
from concourse import mybir
from concourse import masks
from concourse.bass_utils import run_bass_kernel_spmd

F32 = mybir.dt.float32
BF16 = mybir.dt.bfloat16
FP16 = mybir.dt.float16

D = 2048          # feature dim
T = 16384         # total timesteps
N_CORES = 8
TS = T // N_CORES  # timesteps per core = 2048
KS = D // N_CORES  # W1 contraction slice per core = 256


def build_kernel(D=D, TS=TS, KS=KS, n_cores=N_CORES, debug=False):
    DT = D // 128     # d-tiles of 128
    KT = D // 128     # k-tiles of 128
    TC = TS // 512    # t-chunks of 512
    IT = TS // 128    # t-tiles of 128
    GJ = min(4, DT)   # dj per pass-2 group (one 512-wide d-chunk)
    NG = DT // GJ     # number of pass-2 groups
    THW = min(1024, TS)  # pass-2 t-half width
    NTH = TS // THW
    N_CORES_ = n_cores

    nc = bacc.Bacc(None, target_bir_lowering=False, debug=debug, num_devices=N_CORES_)

    # Per-core inputs (see make_in_maps for host-side layouts)
    valsT = nc.dram_tensor("valsT", [D, TS], FP16, kind="ExternalInput")
    w2t = nc.dram_tensor("w2t", [DT, 128, KT * 128], FP16, kind="ExternalInput")
    w1t_d = nc.dram_tensor("w1t_d", [KT, 128, D], FP16, kind="ExternalInput")
    qfull = nc.dram_tensor("qfull", [D], F32, kind="ExternalInput")
    w1b = nc.dram_tensor("w1b", [D], F32, kind="ExternalInput")
    w2b = nc.dram_tensor("w2b", [D], F32, kind="ExternalInput")
    vvec = nc.dram_tensor("vvec", [D], F32, kind="ExternalInput")
    out = nc.dram_tensor("out", [TS, D], F32, kind="ExternalOutput")

    with tile.TileContext(nc) as tc:
        with (
            tc.tile_pool(name="const", bufs=1) as const_pool,
            tc.tile_pool(name="vt", bufs=1) as vt_pool,
            tc.tile_pool(name="e", bufs=1) as e_pool,
            tc.tile_pool(name="w2tb", bufs=2) as w2tb_pool,
            tc.tile_pool(name="st", bufs=2) as st_pool,
            tc.tile_pool(name="outT", bufs=6) as outT_pool,
            tc.tile_pool(name="osb", bufs=10) as osb_pool,
            tc.tile_pool(name="psum", bufs=6, space="PSUM") as psum_pool,
            tc.tile_pool(name="psumT", bufs=2, space="PSUM") as psumT_pool,
            tc.tile_pool(name="dram", bufs=1, space="DRAM") as dram_pool,
        ):
            # ---------------- constants / small vectors ----------------
            qbv = const_pool.tile([128, DT], F32)    # qb[d] laid out [p, dj]
            vv = const_pool.tile([128, DT], F32)     # v[d]
            rv2 = const_pool.tile([128, DT], F32)    # 2^14 / S[d]
            Sloc = const_pool.tile([128, DT], F32)   # local sum-exp
            b1v = const_pool.tile([128, DT], F32)
            b2v = const_pool.tile([128, DT], F32)
            acc = const_pool.tile([128, DT * TC], F32)  # per (dj, tc) exp-sums
            ident16 = const_pool.tile([128, 128], FP16)
            ones1 = const_pool.tile([1, 128], F32)
            qs1 = const_pool.tile([1, KS], F32)
            qpart = const_pool.tile([128, DT], F32)  # local q_proj partial

            masks.make_identity(nc, ident16[:, :])
            nc.vector.memset(ones1[:, :], 1.0)

            # dram bounce buffers for the sum-exp collectives (two halves)
            DH = (3 * DT // 4) if DT >= 4 else DT
            s_in_a = dram_pool.tile([128, DH], F32)
            s_out_a = dram_pool.tile([128, DH], F32)
            if DH < DT:
                s_in_b = dram_pool.tile([128, DT - DH], F32)
                s_out_b = dram_pool.tile([128, DT - DH], F32)

            # ---------------- warmup collective (absorbs ncfw first-use) ----
            wu_in = dram_pool.tile([1, 32], F32, name="wu_in")
            wu_out = dram_pool.tile([1, 32], F32, name="wu_out")
            wuz = const_pool.tile([1, 32], F32)
            nc.vector.memset(wuz[:, :], 0.0)
            nc.gpsimd.dma_start(wu_in[:, :], wuz[:, :])
            nc.gpsimd.collective_compute(
                "AllReduce", mybir.AluOpType.add,
                replica_groups=[list(range(N_CORES_))],
                ins=[wu_in.opt()], outs=[wu_out.opt()],
            )

            # ---------------- q-projection matvec on PE (replicated) -------
            # q_proj row = sum_kt q_col[kt].T @ W1T[kt]; W1T streams in fp16,
            # filling the PE while VT loads.  Then transpose the row into the
            # per-partition [p, dj] layout via 16 tiny PE transposes.
            qcol = const_pool.tile([128, KT], F32)   # q in [p, kt] layout
            qcol16 = const_pool.tile([128, KT], FP16)
            qrow = const_pool.tile([1, D], F32)      # q_proj as a row
            nc.gpsimd.dma_start(qcol[:, :], qfull[:].rearrange("(kt p) -> p kt", p=128))
            nc.vector.tensor_copy(qcol16[:, :], qcol[:, :])
            QW = min(512, D)
            QDC = D // QW
            with tc.tile_pool(name="w1pool", bufs=4) as w1_pool:
                pq_tiles = [psum_pool.tile([1, QW], F32, name=f"pq{dc}", tag="ps")
                            for dc in range(QDC)]
                for kt in range(KT):
                    w1tile = w1_pool.tile([128, D], FP16, tag="w1t")
                    eng = nc.sync if kt % 2 == 0 else nc.scalar
                    eng.dma_start(w1tile[:, :], w1t_d[kt, :, :])
                    for dc in range(QDC):
                        nc.tensor.matmul(
                            pq_tiles[dc][:, :], qcol16[:, kt:kt + 1],
                            w1tile[:, dc * QW:(dc + 1) * QW],
                            start=(kt == 0), stop=(kt == KT - 1))
                for dc in range(QDC):
                    nc.scalar.copy(qrow[:, dc * QW:(dc + 1) * QW], pq_tiles[dc][:, :])
            pqt = psumT_pool.tile([128, DT], F32, name="pqt", tag="pT")
            for dj in range(DT):
                nc.tensor.transpose(
                    pqt[:, dj:dj + 1],
                    qrow[:, dj * 128:(dj + 1) * 128], ones1[:, 0:1])
            nc.scalar.copy(qbv[:, :], pqt[:, :])

            # biases / v in [p, dj] layout: elem (p, j) <- dram[128j + p]
            nc.gpsimd.dma_start(b1v[:, :], w1b[:].rearrange("(j p) -> p j", p=128))
            nc.gpsimd.dma_start(b2v[:, :], w2b[:].rearrange("(j p) -> p j", p=128))
            nc.gpsimd.dma_start(vv[:, :], vvec[:].rearrange("(j p) -> p j", p=128))
            nc.vector.tensor_add(b1v[:, :], b1v[:, :], b2v[:, :])
            nc.vector.tensor_add(qbv[:, :], qbv[:, :], b1v[:, :])

            # ---------------- VT resident load (bf16) -------------------
            # vt[kt][p, t] = values_s[t, 128*kt + p]
            vt_tiles = []
            VH = TS // 2
            for kt in range(KT):
                vt = vt_pool.tile([128, TS], FP16, name=f"vt{kt}")
                vt_tiles.append(vt)
            for half in range(2):
                for kt in range(KT):
                    eng = nc.sync if kt % 2 == 0 else nc.scalar
                    eng.dma_start(
                        vt_tiles[kt][:, half * VH:(half + 1) * VH],
                        valsT[kt * 128:(kt + 1) * 128, half * VH:(half + 1) * VH])

            # ---------------- pass 1: matmul + tanh + exp ---------------
            e_tiles = []
            for dj in range(DT):
                e_tiles.append(e_pool.tile([128, TS], FP16, name=f"e{dj}"))

            ndma_state = [0]

            def emit_group(g):
                # pass-2 pipeline for dj group g (GJ d-tiles, one 512-wide
                # output chunk): scale e by 2^14/S, multiply with resident
                # fp16 VT, transpose on TensorE, descale-evacuate on ScalarE.
                for th in range(NTH):
                    oT = []
                    for jj in range(GJ):
                        dj = g * GJ + jj
                        if th == 0:
                            nc.vector.tensor_scalar(
                                out=e_tiles[dj][:, :], in0=e_tiles[dj][:, :],
                                scalar1=rv2[:, dj:dj + 1], scalar2=None,
                                op0=mybir.AluOpType.mult)
                        ot = outT_pool.tile([128, THW], FP16, tag="oT", name="ot")
                        nc.vector.tensor_mul(
                            ot[:, :],
                            e_tiles[dj][:, th * THW:(th + 1) * THW],
                            vt_tiles[dj][:, th * THW:(th + 1) * THW])
                        oT.append(ot)
                    for itl in range(THW // 128):
                        it = th * (THW // 128) + itl
                        pst = psumT_pool.tile([128, GJ * 128], FP16, tag="pT",
                                              name="pst")
                        for jj in range(GJ):
                            nc.tensor.transpose(
                                pst[:, jj * 128:(jj + 1) * 128],
                                oT[jj][:, itl * 128:(itl + 1) * 128],
                                ident16[:, :],
                            )
                        osb = osb_pool.tile([128, GJ * 128], F32, name="osb")
                        nc.scalar.activation(
                            osb[:, :], pst[:, :],
                            mybir.ActivationFunctionType.Copy,
                            bias=0.0, scale=0.00006103515625)
                        eng = nc.sync if ndma_state[0] % 2 == 0 else nc.gpsimd
                        ndma_state[0] += 1
                        eng.dma_start(
                            out[it * 128:(it + 1) * 128,
                                g * GJ * 128:(g + 1) * GJ * 128],
                            osb[:, :])

            # groups whose d-tiles are all < DH can run on the A-half rv2;
            # interleave them with the tail of pass 1 so TensorE/DVE/ScalarE
            # and the output DMA overlap the matmul stream.
            a_groups = [g for g in range(NG) if (g + 1) * GJ <= DH]
            b_groups = [g for g in range(NG) if (g + 1) * GJ > DH]
            interleave_at = {}
            for i, g in enumerate(a_groups):
                interleave_at.setdefault(min(DT - 1, DT - len(a_groups) + i), []).append(g)

            for dj in range(DT):
                w2tb = w2tb_pool.tile([128, KT * 128], FP16)
                nc.sync.dma_start(w2tb[:, :], w2t[dj, :, :])
                ps_tiles = [psum_pool.tile([128, 512], F32, tag="ps", name=f"ps{i}")
                            for i in range(TC)]
                # k OUTER: stationary operand reused TC times; dj==0 streams
                # at VT-DMA pace.
                for kt in range(KT):
                    for tc_i in range(TC):
                        nc.tensor.matmul(
                            ps_tiles[tc_i][:, :],
                            w2tb[:, kt * 128:(kt + 1) * 128],
                            vt_tiles[kt][:, tc_i * 512:(tc_i + 1) * 512],
                            start=(kt == 0),
                            stop=(kt == KT - 1),
                        )
                for tc_i in range(TC):
                    st = st_pool.tile([128, 512], F32)
                    nc.scalar.activation(
                        st[:, :], ps_tiles[tc_i][:, :],
                        mybir.ActivationFunctionType.Tanh,
                        bias=qbv[:, dj:dj + 1], scale=1.0,
                    )
                    nc.scalar.activation(
                        e_tiles[dj][:, tc_i * 512:(tc_i + 1) * 512], st[:, :],
                        mybir.ActivationFunctionType.Exp,
                        bias=0.0, scale=vv[:, dj:dj + 1],
                        accum_out=acc[:, dj * TC + tc_i:dj * TC + tc_i + 1],
                    )
                nc.vector.tensor_reduce(
                    Sloc[:, dj:dj + 1],
                    acc[:, dj * TC:(dj + 1) * TC],
                    axis=mybir.AxisListType.X,
                    op=mybir.AluOpType.add,
                )
                if dj == DH - 1:
                    # A-part sum-exp AllReduce, overlapped with the rest of
                    # pass 1
                    nc.gpsimd.dma_start(s_in_a[:, :], Sloc[:, 0:DH])
                    nc.gpsimd.collective_compute(
                        "AllReduce", mybir.AluOpType.add,
                        replica_groups=[list(range(N_CORES_))],
                        ins=[s_in_a.opt()], outs=[s_out_a.opt()],
                    )
                    nc.gpsimd.dma_start(rv2[:, 0:DH], s_out_a[:, :])
                    nc.vector.tensor_scalar_mul(rv2[:, 0:DH], rv2[:, 0:DH], 0.00006103515625)
                    nc.vector.reciprocal(rv2[:, 0:DH], rv2[:, 0:DH])
                for g in interleave_at.get(dj, []):
                    emit_group(g)

            # ---------------- B-part sum-exp AllReduce ------------------
            if DH < DT:
                nc.gpsimd.dma_start(s_in_b[:, :], Sloc[:, DH:DT])
                nc.gpsimd.collective_compute(
                    "AllReduce", mybir.AluOpType.add,
                    replica_groups=[list(range(N_CORES_))],
                    ins=[s_in_b.opt()], outs=[s_out_b.opt()],
                )
                nc.gpsimd.dma_start(rv2[:, DH:DT], s_out_b[:, :])
                nc.vector.tensor_scalar_mul(rv2[:, DH:DT], rv2[:, DH:DT], 0.00006103515625)
                nc.vector.reciprocal(rv2[:, DH:DT], rv2[:, DH:DT])

            for g in b_groups:
                emit_group(g)

    nc.compile()
    return nc


_NC_CACHE = None


def _get_nc():
    global _NC_CACHE
    if _NC_CACHE is None:
        _NC_CACHE = build_kernel()
    return _NC_CACHE


def make_in_maps(query, values, v, W1_w, W1_b, W2_w, W2_b,
                 D_=None, TS_=None, KS_=None, n_cores=N_CORES):
    import ml_dtypes
    D_ = D_ or D
    TS_ = TS_ or TS
    KS_ = KS_ or KS
    DT_ = D_ // 128
    KT_ = D_ // 128
    # W1T blocked: [kt, p, d] = W1_w[d, 128kt+p]
    w1t_blocked = np.ascontiguousarray(
        W1_w.T.reshape(KT_, 128, D_).astype(np.float16))
    # w2t blocked: B[dj, p, kt, f] = W2_w[128dj+f, 128kt+p]
    w2t_blocked = np.ascontiguousarray(
        W2_w.reshape(DT_, 128, KT_, 128).transpose(0, 3, 2, 1)
        .reshape(DT_, 128, KT_ * 128).astype(np.float16))
    in_maps = []
    for c in range(n_cores):
        vs = np.ascontiguousarray(values[c * TS_:(c + 1) * TS_])
        vsT = np.ascontiguousarray(vs.T.astype(np.float16))
        in_maps.append({
            "valsT": vsT,
            "w2t": w2t_blocked,
            "w1t_d": w1t_blocked,
            "qfull": query,
            "w1b": W1_b,
            "w2b": W2_b,
            "vvec": v,
        })
    return in_maps


def kernel(query, values, v, W1_w, W1_b, W2_w, W2_b, _trace=False, _trace_kwargs=None):
    query = np.asarray(query, np.float32)
    values = np.asarray(values, np.float32)
    v = np.asarray(v, np.float32)
    W1_w = np.asarray(W1_w, np.float32)
    W1_b = np.asarray(W1_b, np.float32)
    W2_w = np.asarray(W2_w, np.float32)
    W2_b = np.asarray(W2_b, np.float32)

    nc = _get_nc()
    in_maps = make_in_maps(query, values, v, W1_w, W1_b, W2_w, W2_b)
    res = run_bass_kernel_spmd(
        nc, in_maps, core_ids=list(range(N_CORES)),
        trace=_trace, **(_trace_kwargs or {}),
    )
    shards = [np.asarray(om["out"], np.float32) for om in res.results]
    out = np.concatenate(shards, axis=0)
    if _trace:
        return out, res
    return out


# revision 30
# speedup vs baseline: 1.0369x; 1.0369x over previous
"""Bahdanau additive-attention kernel for Trainium2, SPMD across 8 NeuronCores.

Reference computation (all fp32):
    q_proj  = query @ W1_w.T + W1_b            # [D]
    v_proj  = values @ W2_w.T + W2_b           # [T, D]
    weights = softmax(tanh(q_proj + v_proj) * v, axis=0)   # over T
    out     = weights * values                 # [T, D]

Sharding: values is split along T across 8 cores (2048 rows each); W2 is
replicated (shipped pre-transposed + pre-blocked in bf16); the q-projection
matvec is sharded over the contraction dim (each core handles 256 columns of
W1) and finished with an AllReduce; the softmax denominator (per-column sum
of exps) is AllReduced.  Logits are bounded in [-0.1, 0.1] (tanh * v with
|v| <= 0.1) so the softmax needs no max-subtraction pass.

Per-core device program:
  - VT (values shard transposed, bf16) resident in SBUF as the moving matmul
    operand; psum tiles are [d=128 part, t=512 free]; the k loop is OUTER so
    the first tiles stream at DMA pace and the stationary operand is reused
    across 4 consecutive matmuls.
  - ScalarE: tanh(psum + qb[d]) then exp(v[d] * x) with accum_out giving the
    per-partition running sum of exps (softmax denominator) for free.
  - e stored fp16 in SBUF.  Pass 2: e *= 1/S[d] (per-partition tensor_scalar,
    in place), outT = e * valuesT(fp32) on DVE, TensorE transposes outT back
    to [t, d], ScalarE evacuates PSUM to SBUF, DMA out.
"""

import numpy as np

import concourse.bacc as bacc
import concourse.bass as bass
import concourse.tile as tile
from concourse import mybir
from concourse import masks
from concourse.bass_utils import run_bass_kernel_spmd

F32 = mybir.dt.float32
BF16 = mybir.dt.bfloat16
FP16 = mybir.dt.float16

D = 2048          # feature dim
T = 16384         # total timesteps
N_CORES = 8
TS = T // N_CORES  # timesteps per core = 2048
KS = D // N_CORES  # W1 contraction slice per core = 256


def build_kernel(D=D, TS=TS, KS=KS, n_cores=N_CORES, debug=False):
    DT = D // 128     # d-tiles of 128
    KT = D // 128     # k-tiles of 128
    TC = TS // 512    # t-chunks of 512
    IT = TS // 128    # t-tiles of 128
    GJ = min(4, DT)   # dj per pass-2 group (one 512-wide d-chunk)
    NG = DT // GJ     # number of pass-2 groups
    THW = min(1024, TS)  # pass-2 t-half width
    NTH = TS // THW
    N_CORES_ = n_cores

    nc = bacc.Bacc(None, target_bir_lowering=False, debug=debug, num_devices=N_CORES_)

    # Per-core inputs (see make_in_maps for host-side layouts)
    valsT = nc.dram_tensor("valsT", [D, TS], FP16, kind="ExternalInput")
    w2t = nc.dram_tensor("w2t", [DT, 128, KT * 128], FP16, kind="ExternalInput")
    w1t_d = nc.dram_tensor("w1t_d", [KT, 128, D], FP16, kind="ExternalInput")
    qfull = nc.dram_tensor("qfull", [D], F32, kind="ExternalInput")
    w1b = nc.dram_tensor("w1b", [D], F32, kind="ExternalInput")
    w2b = nc.dram_tensor("w2b", [D], F32, kind="ExternalInput")
    vvec = nc.dram_tensor("vvec", [D], F32, kind="ExternalInput")
    out = nc.dram_tensor("out", [TS, D], F32, kind="ExternalOutput")

    with tile.TileContext(nc) as tc:
        with (
            tc.tile_pool(name="const", bufs=1) as const_pool,
            tc.tile_pool(name="vt", bufs=1) as vt_pool,
            tc.tile_pool(name="e", bufs=1) as e_pool,
            tc.tile_pool(name="w2tb", bufs=2) as w2tb_pool,
            tc.tile_pool(name="st", bufs=2) as st_pool,
            tc.tile_pool(name="outT", bufs=6) as outT_pool,
            tc.tile_pool(name="osb", bufs=10) as osb_pool,
            tc.tile_pool(name="psum", bufs=6, space="PSUM") as psum_pool,
            tc.tile_pool(name="psumT", bufs=2, space="PSUM") as psumT_pool,
            tc.tile_pool(name="dram", bufs=1, space="DRAM") as dram_pool,
        ):
            # ---------------- constants / small vectors ----------------
            qbv = const_pool.tile([128, DT], F32)    # qb[d] laid out [p, dj]
            vv = const_pool.tile([128, DT], F32)     # v[d]
            rv2 = const_pool.tile([128, DT], F32)    # 2^14 / S[d]
            Sloc = const_pool.tile([128, DT], F32)   # local sum-exp
            b1v = const_pool.tile([128, DT], F32)
            b2v = const_pool.tile([128, DT], F32)
            acc = const_pool.tile([128, DT * TC], F32)  # per (dj, tc) exp-sums
            ident16 = const_pool.tile([128, 128], FP16)
            ones1 = const_pool.tile([1, 128], F32)
            qs1 = const_pool.tile([1, KS], F32)
            qpart = const_pool.tile([128, DT], F32)  # local q_proj partial

            masks.make_identity(nc, ident16[:, :])
            nc.vector.memset(ones1[:, :], 1.0)

            # dram bounce buffers for the sum-exp collectives (two halves)
            DH = (3 * DT // 4) if DT >= 4 else DT
            s_in_a = dram_pool.tile([128, DH], F32)
            s_out_a = dram_pool.tile([128, DH], F32)
            if DH < DT:
                s_in_b = dram_pool.tile([128, DT - DH], F32)
                s_out_b = dram_pool.tile([128, DT - DH], F32)

            # ---------------- warmup collective (absorbs ncfw first-use) ----
            wu_in = dram_pool.tile([1, 32], F32, name="wu_in")
            wu_out = dram_pool.tile([1, 32], F32, name="wu_out")
            wuz = const_pool.tile([1, 32], F32)
            nc.vector.memset(wuz[:, :], 0.0)
            nc.gpsimd.dma_start(wu_in[:, :], wuz[:, :])
            nc.gpsimd.collective_compute(
                "AllReduce", mybir.AluOpType.add,
                replica_groups=[list(range(N_CORES_))],
                ins=[wu_in.opt()], outs=[wu_out.opt()],
            )

            # ---------------- q-projection matvec on PE (replicated) -------
            # q_proj row = sum_kt q_col[kt].T @ W1T[kt]; W1T streams in fp16,
            # filling the PE while VT loads.  Then transpose the row into the
            # per-partition [p, dj] layout via 16 tiny PE transposes.
            qcol = const_pool.tile([128, KT], F32)   # q in [p, kt] layout
            qcol16 = const_pool.tile([128, KT], FP16)
            qrow = const_pool.tile([1, D], F32)      # q_proj as a row
            nc.gpsimd.dma_start(qcol[:, :], qfull[:].rearrange("(kt p) -> p kt", p=128))
            nc.vector.tensor_copy(qcol16[:, :], qcol[:, :])
            QW = min(512, D)
            QDC = D // QW
            with tc.tile_pool(name="w1pool", bufs=4) as w1_pool:
                pq_tiles = [psum_pool.tile([1, QW], F32, name=f"pq{dc}", tag="ps")
                            for dc in range(QDC)]
                for kt in range(KT):
                    w1tile = w1_pool.tile([128, D], FP16, tag="w1t")
                    eng = nc.sync if kt % 2 == 0 else nc.scalar
                    eng.dma_start(w1tile[:, :], w1t_d[kt, :, :])
                    for dc in range(QDC):
                        nc.tensor.matmul(
                            pq_tiles[dc][:, :], qcol16[:, kt:kt + 1],
                            w1tile[:, dc * QW:(dc + 1) * QW],
                            start=(kt == 0), stop=(kt == KT - 1))
                for dc in range(QDC):
                    nc.scalar.copy(qrow[:, dc * QW:(dc + 1) * QW], pq_tiles[dc][:, :])
            pqt = psumT_pool.tile([128, DT], F32, name="pqt", tag="pT")
            for dj in range(DT):
                nc.tensor.transpose(
                    pqt[:, dj:dj + 1],
                    qrow[:, dj * 128:(dj + 1) * 128], ones1[:, 0:1])
            nc.scalar.copy(qbv[:, :], pqt[:, :])

            # biases / v in [p, dj] layout: elem (p, j) <- dram[128j + p]
            nc.gpsimd.dma_start(b1v[:, :], w1b[:].rearrange("(j p) -> p j", p=128))
            nc.gpsimd.dma_start(b2v[:, :], w2b[:].rearrange("(j p) -> p j", p=128))
            nc.gpsimd.dma_start(vv[:, :], vvec[:].rearrange("(j p) -> p j", p=128))
            nc.vector.tensor_add(b1v[:, :], b1v[:, :], b2v[:, :])
            nc.vector.tensor_add(qbv[:, :], qbv[:, :], b1v[:, :])

            # ---------------- VT resident load (bf16) -------------------
            # vt[kt][p, t] = values_s[t, 128*kt + p]
            vt_tiles = []
            VH = TS // 2
            for kt in range(KT):
                vt = vt_pool.tile([128, TS], FP16, name=f"vt{kt}")
                vt_tiles.append(vt)
            for half in range(2):
                for kt in range(KT):
                    eng = nc.sync if kt % 2 == 0 else nc.scalar
                    eng.dma_start(
                        vt_tiles[kt][:, half * VH:(half + 1) * VH],
                        valsT[kt * 128:(kt + 1) * 128, half * VH:(half + 1) * VH])

            # ---------------- pass 1: matmul + tanh + exp ---------------
            e_tiles = []
            for dj in range(DT):
                e_tiles.append(e_pool.tile([128, TS], FP16, name=f"e{dj}"))

            ndma_state = [0]

            def emit_group(g):
                # pass-2 pipeline for dj group g (GJ d-tiles, one 512-wide
                # output chunk): scale e by 2^14/S, multiply with resident
                # fp16 VT, transpose on TensorE, descale-evacuate on ScalarE.
                for th in range(NTH):
                    oT = []
                    for jj in range(GJ):
                        dj = g * GJ + jj
                        if th == 0:
                            nc.vector.tensor_scalar(
                                out=e_tiles[dj][:, :], in0=e_tiles[dj][:, :],
                                scalar1=rv2[:, dj:dj + 1], scalar2=None,
                                op0=mybir.AluOpType.mult)
                        ot = outT_pool.tile([128, THW], FP16, tag="oT", name="ot")
                        nc.vector.tensor_mul(
                            ot[:, :],
                            e_tiles[dj][:, th * THW:(th + 1) * THW],
                            vt_tiles[dj][:, th * THW:(th + 1) * THW])
                        oT.append(ot)
                    for itl in range(THW // 128):
                        it = th * (THW // 128) + itl
                        pst = psumT_pool.tile([128, GJ * 128], FP16, tag="pT",
                                              name="pst")
                        for jj in range(GJ):
                            nc.tensor.transpose(
                                pst[:, jj * 128:(jj + 1) * 128],
                                oT[jj][:, itl * 128:(itl + 1) * 128],
                                ident16[:, :],
                            )
                        osb = osb_pool.tile([128, GJ * 128], F32, name="osb")
                        nc.scalar.activation(
                            osb[:, :], pst[:, :],
                            mybir.ActivationFunctionType.Copy,
                            bias=0.0, scale=0.00006103515625)
                        ndma_state[0] += 1
                        nc.sync.dma_start(
                            out[it * 128:(it + 1) * 128,
                                g * GJ * 128:(g + 1) * GJ * 128],
                            osb[:, :])

            # groups whose d-tiles are all < DH can run on the A-half rv2;
            # interleave them with the tail of pass 1 so TensorE/DVE/ScalarE
            # and the output DMA overlap the matmul stream.
            a_groups = [g for g in range(NG) if (g + 1) * GJ <= DH]
            b_groups = [g for g in range(NG) if (g + 1) * GJ > DH]
            interleave_at = {}
            if a_groups and DT >= 8:
                interleave_at[DT - 3] = [a_groups[0]]
                mid_groups = a_groups[1:]
            else:
                mid_groups = list(a_groups)

            for dj in range(DT):
                w2tb = w2tb_pool.tile([128, KT * 128], FP16)
                nc.sync.dma_start(w2tb[:, :], w2t[dj, :, :])
                ps_tiles = [psum_pool.tile([128, 512], F32, tag="ps", name=f"ps{i}")
                            for i in range(TC)]
                # k OUTER: stationary operand reused TC times; dj==0 streams
                # at VT-DMA pace.
                for kt in range(KT):
                    for tc_i in range(TC):
                        nc.tensor.matmul(
                            ps_tiles[tc_i][:, :],
                            w2tb[:, kt * 128:(kt + 1) * 128],
                            vt_tiles[kt][:, tc_i * 512:(tc_i + 1) * 512],
                            start=(kt == 0),
                            stop=(kt == KT - 1),
                        )
                for tc_i in range(TC):
                    st = st_pool.tile([128, 512], F32)
                    nc.scalar.activation(
                        st[:, :], ps_tiles[tc_i][:, :],
                        mybir.ActivationFunctionType.Tanh,
                        bias=qbv[:, dj:dj + 1], scale=1.0,
                    )
                    nc.scalar.activation(
                        e_tiles[dj][:, tc_i * 512:(tc_i + 1) * 512], st[:, :],
                        mybir.ActivationFunctionType.Exp,
                        bias=0.0, scale=vv[:, dj:dj + 1],
                        accum_out=acc[:, dj * TC + tc_i:dj * TC + tc_i + 1],
                    )
                nc.vector.tensor_reduce(
                    Sloc[:, dj:dj + 1],
                    acc[:, dj * TC:(dj + 1) * TC],
                    axis=mybir.AxisListType.X,
                    op=mybir.AluOpType.add,
                )
                if dj == DH - 1:
                    # A-part sum-exp AllReduce, overlapped with the rest of
                    # pass 1
                    nc.gpsimd.dma_start(s_in_a[:, :], Sloc[:, 0:DH])
                    nc.gpsimd.collective_compute(
                        "AllReduce", mybir.AluOpType.add,
                        replica_groups=[list(range(N_CORES_))],
                        ins=[s_in_a.opt()], outs=[s_out_a.opt()],
                    )
                    nc.gpsimd.dma_start(rv2[:, 0:DH], s_out_a[:, :])
                    nc.vector.tensor_scalar_mul(rv2[:, 0:DH], rv2[:, 0:DH], 0.00006103515625)
                    nc.vector.reciprocal(rv2[:, 0:DH], rv2[:, 0:DH])
                for g in interleave_at.get(dj, []):
                    emit_group(g)

            # ---------------- B-part sum-exp AllReduce ------------------
            # Trigger immediately after dj15's local reduce; run the
            # remaining A-half groups during its ~25us latency; read back and
            # reciprocal only after their DVE work is queued.
            if DH < DT:
                nc.gpsimd.dma_start(s_in_b[:, :], Sloc[:, DH:DT])
                nc.gpsimd.collective_compute(
                    "AllReduce", mybir.AluOpType.add,
                    replica_groups=[list(range(N_CORES_))],
                    ins=[s_in_b.opt()], outs=[s_out_b.opt()],
                )

            for g in mid_groups:
                emit_group(g)

            if DH < DT:
                nc.gpsimd.dma_start(rv2[:, DH:DT], s_out_b[:, :])
                nc.vector.tensor_scalar_mul(rv2[:, DH:DT], rv2[:, DH:DT], 0.00006103515625)
                nc.vector.reciprocal(rv2[:, DH:DT], rv2[:, DH:DT])

            for g in b_groups:
                emit_group(g)

    nc.compile()
    return nc


_NC_CACHE = None


def _get_nc():
    global _NC_CACHE
    if _NC_CACHE is None:
        _NC_CACHE = build_kernel()
    return _NC_CACHE


def make_in_maps(query, values, v, W1_w, W1_b, W2_w, W2_b,
                 D_=None, TS_=None, KS_=None, n_cores=N_CORES):
    import ml_dtypes
    D_ = D_ or D
    TS_ = TS_ or TS
    KS_ = KS_ or KS
    DT_ = D_ // 128
    KT_ = D_ // 128
    # W1T blocked: [kt, p, d] = W1_w[d, 128kt+p]
    w1t_blocked = np.ascontiguousarray(
        W1_w.T.reshape(KT_, 128, D_).astype(np.float16))
    # w2t blocked: B[dj, p, kt, f] = W2_w[128dj+f, 128kt+p]
    w2t_blocked = np.ascontiguousarray(
        W2_w.reshape(DT_, 128, KT_, 128).transpose(0, 3, 2, 1)
        .reshape(DT_, 128, KT_ * 128).astype(np.float16))
    in_maps = []
    for c in range(n_cores):
        vs = np.ascontiguousarray(values[c * TS_:(c + 1) * TS_])
        vsT = np.ascontiguousarray(vs.T.astype(np.float16))
        in_maps.append({
            "valsT": vsT,
            "w2t": w2t_blocked,
            "w1t_d": w1t_blocked,
            "qfull": query,
            "w1b": W1_b,
            "w2b": W2_b,
            "vvec": v,
        })
    return in_maps


def kernel(query, values, v, W1_w, W1_b, W2_w, W2_b, _trace=False, _trace_kwargs=None):
    query = np.asarray(query, np.float32)
    values = np.asarray(values, np.float32)
    v = np.asarray(v, np.float32)
    W1_w = np.asarray(W1_w, np.float32)
    W1_b = np.asarray(W1_b, np.float32)
    W2_w = np.asarray(W2_w, np.float32)
    W2_b = np.asarray(W2_b, np.float32)

    nc = _get_nc()
    in_maps = make_in_maps(query, values, v, W1_w, W1_b, W2_w, W2_b)
    res = run_bass_kernel_spmd(
        nc, in_maps, core_ids=list(range(N_CORES)),
        trace=_trace, **(_trace_kwargs or {}),
    )
    shards = [np.asarray(om["out"], np.float32) for om in res.results]
    out = np.concatenate(shards, axis=0)
    if _trace:
        return out, res
    return out
